# revision 12
# baseline (speedup 1.0000x reference)
"""BitLinear (ternary-weight linear with int8 activation quantization) on 8 trn2 cores.

y = (clip(round(x/x_scale),-128,127) * x_scale) @ (clip(round(w/w_scale),-1,1) * w_scale).T
  x_scale = max(max|x|, eps)/127   (per-tensor)
  w_scale = max(mean|w|, eps)      (per-tensor)

Sharding: tensor-parallel over out_features (11008 = 8 x 1376), x replicated.
The two per-tensor scalar scales are computed host-side (they replicate
trivially); a single device launch quantizes x/w on the fly (exact-integer
bf16 magic rounding) and runs the int8xternary matmul at the bf16 PE roofline.

Emission order: w slice 0 and the four EARLY x blocks are quantized first so
the PE starts at ~25us and never starves; w slices 1/2 quantize while the PE
works through the EARLY blocks' slice-0 matmuls.
"""

import numpy as np
from contextlib import ExitStack

import concourse.bass as bass
import concourse.tile as tile
from concourse import bacc, mybir
from concourse.bass_utils import run_bass_kernel_spmd

# problem shapes (hardcoded per contract)
B, T, I, O = 4, 2048, 4096, 11008
TOK = B * T                  # 8192
N_CORES = 8
O_SH = O // N_CORES          # 1376
EPS = 1e-5
MAGIC = 12582912.0           # 1.5 * 2**23: fp32 add forces round-to-nearest-even int
F32 = mybir.dt.float32
BF16 = mybir.dt.bfloat16

# tiling
TB = 256                     # tokens per streaming block (2 PSUM m-tiles)
NBLK = TOK // TB             # 32
KT = I // 128                # 32 k-tiles
CH = 2                       # k-tiles per x DMA chunk (0.25MB)
NCH = KT // CH               # 4 chunks per block
WCH = 2                      # k-tiles per w prologue chunk
OB = (512, 512, 352)         # out-feature split per PSUM bank (sum = 1376)
OB_OFF = (0, 512, 1024)
EARLY = 5                    # blocks run slice-0-only while w slices 1/2 load


def _build_matmul():
    nc = bacc.Bacc("TRN2", target_bir_lowering=False, debug=False,
                   num_devices=N_CORES)
    xT = nc.dram_tensor("xT", [I, TOK], F32, kind="ExternalInput").ap()
    wT = nc.dram_tensor("wT", [I, O_SH], F32, kind="ExternalInput").ap()
    consts = nc.dram_tensor("consts", [1, 8], F32, kind="ExternalInput").ap()
    out = nc.dram_tensor("out", [TOK, O_SH], F32, kind="ExternalOutput").ap()

    xTr = xT.rearrange("(kt p) t -> p kt t", p=128)   # [128, KT, TOK]
    wTr = wT.rearrange("(kt p) o -> p kt o", p=128)   # [128, KT, O_SH]

    with tile.TileContext(nc) as tc:
        with ExitStack() as ctx:
            const_pool = ctx.enter_context(tc.tile_pool(name="const", bufs=1))
            wq_pool = ctx.enter_context(tc.tile_pool(name="wq", bufs=1))
            stage = ctx.enter_context(tc.tile_pool(name="stage", bufs=2))
            rnd = ctx.enter_context(tc.tile_pool(name="rnd", bufs=2))
            wstage = ctx.enter_context(tc.tile_pool(name="wstage", bufs=2))
            wrnd = ctx.enter_context(tc.tile_pool(name="wrnd", bufs=2))
            xq_pool = ctx.enter_context(tc.tile_pool(name="xq", bufs=5))
            out_pool = ctx.enter_context(tc.tile_pool(name="out", bufs=3))
            psum = ctx.enter_context(tc.tile_pool(name="psum", bufs=6, space="PSUM"))

            sb_c = const_pool.tile([128, 8], F32)
            nc.sync.dma_start(sb_c[:], consts.to_broadcast((128, 8)))
            inv_w = sb_c[:, 0:1]
            inv_x = sb_c[:, 1:2]
            out_scale = sb_c[:, 2:3]

            # SBUF-resident ternarized weight shard, bf16 [128, KT, O_SH]
            wq = wq_pool.tile([128, KT, O_SH], BF16)

            def quant_w_slice(b):
                o0, ow = OB_OFF[b], OB[b]
                for c in range(KT // WCH):
                    wf = wstage.tile([128, WCH, ow], F32, tag="wstage",
                                     name=f"wf{b}_{c}")
                    nc.sync.dma_start(wf[:], wTr[:, c * WCH:(c + 1) * WCH,
                                              o0:o0 + ow])
                    wr_ = wrnd.tile([128, WCH, ow], F32, tag="wrnd",
                                    name=f"wr{b}_{c}")
                    # round(w * inv_w) in magic space (ACT: out = in*scale + bias)
                    nc.scalar.activation(wr_[:], wf[:],
                                         mybir.ActivationFunctionType.Copy,
                                         bias=MAGIC, scale=inv_w)
                    # clip to [-1, 1] in magic space, subtract magic, cast bf16
                    nc.vector.tensor_scalar(wr_[:], wr_[:], MAGIC + 1.0, MAGIC - 1.0,
                                            op0=mybir.AluOpType.min,
                                            op1=mybir.AluOpType.max)
                    nc.vector.tensor_scalar(
                        wq[:, c * WCH:(c + 1) * WCH, o0:o0 + ow],
                        wr_[:], -MAGIC, None, op0=mybir.AluOpType.add)

            xq_tiles = {}

            def quant_x_block(tb):
                t0 = tb * TB
                xq = xq_pool.tile([128, KT, TB], BF16, tag="xq", name=f"xq{tb}")
                xq_tiles[tb] = xq
                for c in range(NCH):
                    xf = stage.tile([128, CH, TB], F32, tag="stage",
                                    name=f"xf{tb}_{c}")
                    nc.sync.dma_start(xf[:], xTr[:, c * CH:(c + 1) * CH,
                                              t0:t0 + TB])
                    xr_ = rnd.tile([128, CH, TB], F32, tag="rnd",
                                   name=f"xr{tb}_{c}")
                    nc.scalar.activation(xr_[:], xf[:],
                                         mybir.ActivationFunctionType.Copy,
                                         bias=MAGIC, scale=inv_x)
                    # no clip needed: |x|/x_scale <= 127 by construction
                    nc.vector.tensor_scalar(
                        xq[:, c * CH:(c + 1) * CH, :],
                        xr_[:], -MAGIC, None, op0=mybir.AluOpType.add)

            def mm_j(tb, j, bs):
                """matmul groups for m-tile j of block tb, psum banks bs."""
                xq = xq_tiles[tb]
                ps = {}
                for b in bs:
                    ps[b] = psum.tile([128, 512], F32, tag="ps",
                                      name=f"ps{tb}_{j}_{b}")
                    for k in range(KT):
                        nc.tensor.matmul(ps[b][:, :OB[b]],
                                         xq[:, k, j * 128:(j + 1) * 128],
                                         wq[:, k, OB_OFF[b]:OB_OFF[b] + OB[b]],
                                         start=(k == 0), stop=(k == KT - 1))
                t0 = tb * TB + j * 128
                for b in bs:
                    ob = out_pool.tile([128, 512], F32, tag="ob",
                                       name=f"ob{tb}_{j}_{b}")
                    nc.scalar.mul(ob[:, :OB[b]], ps[b][:, :OB[b]], out_scale)
                    nc.sync.dma_start(
                        out[t0:t0 + 128, OB_OFF[b]:OB_OFF[b] + OB[b]],
                        ob[:, :OB[b]])

            # emission order tuned so the DMA queue feeds PE without stalls:
            # w slice 0 + first x blocks, then remaining w slices interleaved;
            # the first EARLY blocks run slice 0 only while slices 1/2 load.
            quant_w_slice(0)
            quant_x_block(0)
            quant_x_block(1)
            quant_x_block(2)
            quant_w_slice(1)
            quant_x_block(3)
            quant_x_block(4)
            quant_w_slice(2)
            for b in range(3):
                for tb in range(EARLY):
                    for j in range(TB // 128):
                        mm_j(tb, j, [b])
            for tb in range(EARLY, NBLK):
                quant_x_block(tb)
                for j in range(TB // 128):
                    mm_j(tb, j, [0, 1, 2])
    nc.compile()
    return nc


_cache = {}


def _get_nc():
    if "B" not in _cache:
        _cache["B"] = _build_matmul()
    return _cache["B"]


def _run(nc, in_maps, core_ids):
    try:
        return run_bass_kernel_spmd(nc, in_maps, core_ids)
    except Exception:
        import time as _t
        _t.sleep(10)  # transient tunnel/device hiccups recover on retry
        return run_bass_kernel_spmd(nc, in_maps, core_ids)


def kernel(x: np.ndarray, weight: np.ndarray) -> np.ndarray:
    ncB = _get_nc()
    core_ids = list(range(N_CORES))

    x = np.asarray(x)
    weight = np.asarray(weight)
    assert x.shape == (B, T, I) and weight.shape == (O, I), (x.shape, weight.shape)
    x_flat = np.ascontiguousarray(x.reshape(TOK, I), dtype=np.float32)
    weight = np.ascontiguousarray(weight, dtype=np.float32)

    # per-tensor scalar scales (host: they replicate trivially across cores)
    absmax = np.float32(np.max(np.abs(x_flat)))
    wmean = np.float32(np.float32(np.sum(np.abs(weight), dtype=np.float64)) /
                       np.float32(O * I))
    x_scale = np.float32(max(absmax, np.float32(EPS))) / np.float32(127.0)
    w_scale = np.float32(max(wmean, np.float32(EPS)))
    consts = np.zeros((1, 8), dtype=np.float32)
    consts[0, 0] = np.float32(1.0) / w_scale
    consts[0, 1] = np.float32(1.0) / x_scale
    consts[0, 2] = x_scale * w_scale

    # single launch: quantize + exact-integer bf16 matmul, TP over out_features
    xT = np.ascontiguousarray(x_flat.T)               # [I, TOK]
    wTf = weight.T                                    # [I, O] view
    in_B = [{
        "xT": xT,
        "wT": np.ascontiguousarray(wTf[:, i * O_SH:(i + 1) * O_SH]),
        "consts": consts,
    } for i in range(N_CORES)]
    resB = _run(ncB, in_B, core_ids)
    out = np.concatenate([resB.results[i]["out"] for i in range(N_CORES)], axis=1)
    return out.reshape(B, T, O)


# revision 13
# speedup vs baseline: 1.0950x; 1.0950x over previous
"""BitLinear (ternary-weight linear with int8 activation quantization) on 8 trn2 cores.

y = (clip(round(x/x_scale),-128,127) * x_scale) @ (clip(round(w/w_scale),-1,1) * w_scale).T
  x_scale = max(max|x|, eps)/127   (per-tensor)
  w_scale = max(mean|w|, eps)      (per-tensor)

Sharding: tensor-parallel over out_features (11008 = 8 x 1376), x replicated.
The two per-tensor scalar scales are computed host-side (they replicate
trivially); a single device launch quantizes x/w on the fly (exact-integer
bf16 magic rounding) and runs the int8xternary matmul at the bf16 PE roofline.

Emission order: w slice 0 and the four EARLY x blocks are quantized first so
the PE starts at ~25us and never starves; w slices 1/2 quantize while the PE
works through the EARLY blocks' slice-0 matmuls.
"""

import numpy as np
from contextlib import ExitStack

import concourse.bass as bass
import concourse.tile as tile
from concourse import bacc, mybir
from concourse.bass_utils import run_bass_kernel_spmd

# problem shapes (hardcoded per contract)
B, T, I, O = 4, 2048, 4096, 11008
TOK = B * T                  # 8192
N_CORES = 8
O_SH = O // N_CORES          # 1376
EPS = 1e-5
MAGIC = 12582912.0           # 1.5 * 2**23: fp32 add forces round-to-nearest-even int
F32 = mybir.dt.float32
BF16 = mybir.dt.bfloat16

# tiling
TB = 256                     # tokens per streaming block (2 PSUM m-tiles)
NBLK = TOK // TB             # 32
KT = I // 128                # 32 k-tiles
CH = 8                       # k-tiles per x DMA chunk (CH*TB*4B*128 = 1MB)
NCH = KT // CH               # 4 chunks per block
WCH = 2                      # k-tiles per w prologue chunk
OB = (512, 512, 352)         # out-feature split per PSUM bank (sum = 1376)
OB_OFF = (0, 512, 1024)
EARLY = 4                    # blocks run slice-0-only while w slices 1/2 load


def _build_matmul():
    nc = bacc.Bacc("TRN2", target_bir_lowering=False, debug=False,
                   num_devices=N_CORES)
    xT = nc.dram_tensor("xT", [I, TOK], F32, kind="ExternalInput").ap()
    wT = nc.dram_tensor("wT", [I, O_SH], F32, kind="ExternalInput").ap()
    consts = nc.dram_tensor("consts", [1, 8], F32, kind="ExternalInput").ap()
    out = nc.dram_tensor("out", [TOK, O_SH], F32, kind="ExternalOutput").ap()

    xTr = xT.rearrange("(kt p) t -> p kt t", p=128)   # [128, KT, TOK]
    wTr = wT.rearrange("(kt p) o -> p kt o", p=128)   # [128, KT, O_SH]

    with tile.TileContext(nc) as tc:
        with ExitStack() as ctx:
            const_pool = ctx.enter_context(tc.tile_pool(name="const", bufs=1))
            wq_pool = ctx.enter_context(tc.tile_pool(name="wq", bufs=1))
            stage = ctx.enter_context(tc.tile_pool(name="stage", bufs=2))
            rnd = ctx.enter_context(tc.tile_pool(name="rnd", bufs=2))
            wstage = ctx.enter_context(tc.tile_pool(name="wstage", bufs=2))
            wrnd = ctx.enter_context(tc.tile_pool(name="wrnd", bufs=2))
            xq_pool = ctx.enter_context(tc.tile_pool(name="xq", bufs=4))
            out_pool = ctx.enter_context(tc.tile_pool(name="out", bufs=4))
            psum = ctx.enter_context(tc.tile_pool(name="psum", bufs=6, space="PSUM"))

            sb_c = const_pool.tile([128, 8], F32)
            nc.sync.dma_start(sb_c[:], consts.to_broadcast((128, 8)))
            inv_w = sb_c[:, 0:1]
            inv_x = sb_c[:, 1:2]
            out_scale = sb_c[:, 2:3]

            # SBUF-resident ternarized weight shard, bf16 [128, KT, O_SH]
            wq = wq_pool.tile([128, KT, O_SH], BF16)

            def quant_w_slice(b):
                o0, ow = OB_OFF[b], OB[b]
                for c in range(KT // WCH):
                    wf = wstage.tile([128, WCH, ow], F32, tag="wstage",
                                     name=f"wf{b}_{c}")
                    nc.sync.dma_start(wf[:], wTr[:, c * WCH:(c + 1) * WCH,
                                              o0:o0 + ow])
                    wr_ = wrnd.tile([128, WCH, ow], F32, tag="wrnd",
                                    name=f"wr{b}_{c}")
                    # round(w * inv_w) in magic space (ACT: out = in*scale + bias)
                    nc.scalar.activation(wr_[:], wf[:],
                                         mybir.ActivationFunctionType.Copy,
                                         bias=MAGIC, scale=inv_w)
                    # clip to [-1, 1] in magic space, subtract magic, cast bf16
                    nc.vector.tensor_scalar(wr_[:], wr_[:], MAGIC + 1.0, MAGIC - 1.0,
                                            op0=mybir.AluOpType.min,
                                            op1=mybir.AluOpType.max)
                    nc.vector.tensor_scalar(
                        wq[:, c * WCH:(c + 1) * WCH, o0:o0 + ow],
                        wr_[:], -MAGIC, None, op0=mybir.AluOpType.add)

            xq_tiles = {}

            def quant_x_block(tb):
                t0 = tb * TB
                xq = xq_pool.tile([128, KT, TB], BF16, tag="xq", name=f"xq{tb}")
                xq_tiles[tb] = xq
                for c in range(NCH):
                    xf = stage.tile([128, CH, TB], F32, tag="stage",
                                    name=f"xf{tb}_{c}")
                    nc.sync.dma_start(xf[:], xTr[:, c * CH:(c + 1) * CH,
                                              t0:t0 + TB])
                    xr_ = rnd.tile([128, CH, TB], F32, tag="rnd",
                                   name=f"xr{tb}_{c}")
                    nc.scalar.activation(xr_[:], xf[:],
                                         mybir.ActivationFunctionType.Copy,
                                         bias=MAGIC, scale=inv_x)
                    # no clip needed: |x|/x_scale <= 127 by construction
                    nc.vector.tensor_scalar(
                        xq[:, c * CH:(c + 1) * CH, :],
                        xr_[:], -MAGIC, None, op0=mybir.AluOpType.add)

            def mm_j(tb, j, bs):
                """matmul groups for m-tile j of block tb, psum banks bs."""
                xq = xq_tiles[tb]
                ps = {}
                for b in bs:
                    ps[b] = psum.tile([128, 512], F32, tag="ps",
                                      name=f"ps{tb}_{j}_{b}")
                    for k in range(KT):
                        nc.tensor.matmul(ps[b][:, :OB[b]],
                                         xq[:, k, j * 128:(j + 1) * 128],
                                         wq[:, k, OB_OFF[b]:OB_OFF[b] + OB[b]],
                                         start=(k == 0), stop=(k == KT - 1))
                t0 = tb * TB + j * 128
                for b in bs:
                    ob = out_pool.tile([128, 512], F32, tag="ob",
                                       name=f"ob{tb}_{j}_{b}")
                    nc.scalar.mul(ob[:, :OB[b]], ps[b][:, :OB[b]], out_scale)
                    nc.sync.dma_start(
                        out[t0:t0 + 128, OB_OFF[b]:OB_OFF[b] + OB[b]],
                        ob[:, :OB[b]])

            # emission order tuned so the DMA queue feeds PE without stalls:
            # w slice 0 + first x blocks, then remaining w slices interleaved;
            # the first EARLY blocks run slice 0 only while slices 1/2 load.
            quant_w_slice(0)
            quant_x_block(0)
            quant_x_block(1)
            quant_x_block(2)
            quant_w_slice(1)
            quant_x_block(3)
            quant_w_slice(2)
            for b in range(3):
                for tb in range(EARLY):
                    for j in range(TB // 128):
                        mm_j(tb, j, [b])
            for tb in range(EARLY, NBLK):
                quant_x_block(tb)
                for j in range(TB // 128):
                    mm_j(tb, j, [0, 1, 2])
    nc.compile()
    return nc


_cache = {}


def _get_nc():
    if "B" not in _cache:
        _cache["B"] = _build_matmul()
    return _cache["B"]


def _run(nc, in_maps, core_ids):
    try:
        return run_bass_kernel_spmd(nc, in_maps, core_ids)
    except Exception:
        import time as _t
        _t.sleep(10)  # transient tunnel/device hiccups recover on retry
        return run_bass_kernel_spmd(nc, in_maps, core_ids)


def kernel(x: np.ndarray, weight: np.ndarray) -> np.ndarray:
    ncB = _get_nc()
    core_ids = list(range(N_CORES))

    x = np.asarray(x)
    weight = np.asarray(weight)
    assert x.shape == (B, T, I) and weight.shape == (O, I), (x.shape, weight.shape)
    x_flat = np.ascontiguousarray(x.reshape(TOK, I), dtype=np.float32)
    weight = np.ascontiguousarray(weight, dtype=np.float32)

    # per-tensor scalar scales (host: they replicate trivially across cores)
    absmax = np.float32(np.max(np.abs(x_flat)))
    wmean = np.float32(np.float32(np.sum(np.abs(weight), dtype=np.float64)) /
                       np.float32(O * I))
    x_scale = np.float32(max(absmax, np.float32(EPS))) / np.float32(127.0)
    w_scale = np.float32(max(wmean, np.float32(EPS)))
    consts = np.zeros((1, 8), dtype=np.float32)
    consts[0, 0] = np.float32(1.0) / w_scale
    consts[0, 1] = np.float32(1.0) / x_scale
    consts[0, 2] = x_scale * w_scale

    # single launch: quantize + exact-integer bf16 matmul, TP over out_features
    xT = np.ascontiguousarray(x_flat.T)               # [I, TOK]
    wTf = weight.T                                    # [I, O] view
    in_B = [{
        "xT": xT,
        "wT": np.ascontiguousarray(wTf[:, i * O_SH:(i + 1) * O_SH]),
        "consts": consts,
    } for i in range(N_CORES)]
    resB = _run(ncB, in_B, core_ids)
    out = np.concatenate([resB.results[i]["out"] for i in range(N_CORES)], axis=1)
    return out.reshape(B, T, O)
